# revision 26
# baseline (speedup 1.0000x reference)
"""Causal depthwise conv1d (K=4) + SiLU, sharded over 8 NeuronCores.

Full shapes: x [4, 8192, 2048] f32, weight [2048, 4] f32 -> y [4, 8192, 2048] f32.

Strategy: tensor-parallel over the hidden/channel dim (fully channel
independent, no halo exchange). Each core gets 256 channels, reorganized
host-side to channel-major [B*256, 3+S] (3 leading zero columns provide the
causal padding) so the conv runs along the free dim with channels on SBUF
partitions.

Compute: all 4 taps run on the TensorEngine as float32r diagonal-matrix
matmuls accumulating in PSUM (psum[c,t] += w_i[c] * x[c, t-3+i] via
diag(w_i) @ x_shifted). The diagonal weight matrices are built on-device
(gpsimd affine_select identity mask x per-partition tap scalar on DVE). DVE
rounds each input tile fp32 -> fp32r (the fp32r matmul contract requires
rounded producers); ACT applies SiLU straight out of PSUM and triggers the
output DMA on its own HWDGE ring, with inputs streaming on SP's ring.

Raw bass (no Tile framework): the installed walrus codegen only accepts one
sync wait per compute instruction, so all synchronization is explicit wait_ge
sequencer instructions. Per-buffer-slot DMA semaphores keep concurrent DMA
completion increments unambiguous. Sem increments fire at instruction
completion, but the sequencer runs ahead, so consumers of an engine's result
always gate on that completion increment (including same-engine self-waits
before DMA triggers).
"""

import contextlib

import numpy as np

B, S, H, K = 4, 8192, 2048, 4
N_CORES = 8
HC = H // N_CORES          # 256 channels per core
ROWS = B * HC              # 1024 rows per core, row r = b*HC + c
NU = ROWS // 128           # 8 partition units
T = 2048                   # token tile
NT = S // T
NTILES = NU * NT           # 32
NB = 6                     # buffers per tile kind
NC_CHUNK = 512             # one PSUM bank of fp32
NCHUNKS = T // NC_CHUNK

_last_results = None       # test harness introspection (exec_time_ns etc.)
_ACT_FUNC = "Silu"         # sim override hook (CoreSim lacks Silu)


def _build_program():
    from concourse import bass, mybir

    f32 = mybir.dt.float32
    f32r = mybir.dt.float32r
    AF = mybir.ActivationFunctionType

    nc = bass.Bass()
    # x arrives with 3 leading zero columns (causal padding): [ROWS, 3+S]
    x_d = nc.declare_dram_parameter("x", [ROWS, S + 3], f32, isOutput=False)
    w_d = nc.declare_dram_parameter("w", [128, NU * K + 1], f32, isOutput=False)
    y_d = nc.declare_dram_parameter("y", [ROWS, S], f32, isOutput=True)

    with contextlib.ExitStack() as st:
        wt = st.enter_context(nc.sbuf_tensor("wt", [128, NU * K + 1], f32))
        eye = st.enter_context(nc.sbuf_tensor("eye", [128, 128], f32))
        wtr = st.enter_context(nc.sbuf_tensor("wtr", [128, NU * K * 128], f32r))
        xts = [
            st.enter_context(nc.sbuf_tensor(f"xt{i}", [128, T + 3], f32))
            for i in range(NB)
        ]
        xrs = [
            st.enter_context(nc.sbuf_tensor(f"xr{i}", [128, T + 3], f32r))
            for i in range(NB)
        ]
        yts = [
            st.enter_context(nc.sbuf_tensor(f"yt{i}", [128, T], f32))
            for i in range(NB)
        ]
        pss = [
            st.enter_context(nc.psum_tensor(f"ps{i}", [128, T], f32))
            for i in range(2)
        ]
        zb = wt[:, NU * K : NU * K + 1]           # zeros column (Silu bias)

        def wdiag(k, i):
            u = k // NT
            c0 = (u * K + i) * 128
            return wtr[:, c0 : c0 + 128]

        def x_rows(k):
            r0 = (k // NT) * 128
            return r0, r0 + 128

        with (
            nc.Block() as block,
            nc.semaphore("wsem") as wsem,
            nc.semaphore("esem") as esem,
            nc.semaphore("act") as act,
            nc.semaphore("dve") as dve,
            nc.semaphore("pe") as pe,
            contextlib.ExitStack() as sems,
        ):
            din = [
                sems.enter_context(nc.semaphore(f"din{i}")) for i in range(NB)
            ]
            dout = [
                sems.enter_context(nc.semaphore(f"dout{i}")) for i in range(NB)
            ]

            @block.sync
            def _(sync):
                sync.dma_start(out=wt[:, :], in_=w_d[:, :]).then_inc(wsem, 16)
                for k in range(NTILES):
                    r0, r1 = x_rows(k)
                    t0 = (k % NT) * T
                    if k >= NB:
                        # xt slot free once DVE rounded tile k-NB out of it
                        sync.wait_ge(dve, k - NB + 2)
                    # padded coords: window [t0-3, t0+T) = x_d cols [t0, t0+T+3)
                    sync.dma_start(
                        out=xts[k % NB][:, :],
                        in_=x_d[r0:r1, t0 : t0 + T + 3],
                    ).then_inc(din[k % NB], 16)

            @block.gpsimd
            def _(gpsimd):
                # identity mask for the diagonal weight build (affine_select
                # only exists on gpsimd)
                gpsimd.memset(eye[:, :], 1.0)
                gpsimd.affine_select(
                    out=eye[:, :], in_=eye[:, :],
                    pattern=[[1, 128]], base=0, channel_multiplier=-1,
                    compare_op=mybir.AluOpType.is_equal, fill=0.0,
                ).then_inc(esem)

            @block.vector
            def _(vector):
                # build the 32 diagonal weight matrices on-device
                vector.wait_ge(wsem, 16)
                vector.wait_ge(esem, 1)
                for u in range(NU):
                    for i in range(K):
                        mul = vector.tensor_scalar_mul(
                            wtr[:, (u * K + i) * 128 : (u * K + i + 1) * 128],
                            eye[:, :],
                            wt[:, u * K + i : u * K + i + 1],
                        )
                mul.then_inc(dve)
                # per-tile fp32 -> fp32r rounding
                for k in range(NTILES):
                    vector.wait_ge(din[k % NB], 16 * (k // NB + 1))
                    if k >= NB:
                        # xr slot free once PE consumed tile k-NB
                        vector.wait_ge(pe, k - NB + 1)
                    vector.tensor_copy(
                        out=xrs[k % NB][:, :], in_=xts[k % NB][:, :]
                    ).then_inc(dve)

            @block.tensor
            def _(tensor):
                for k in range(NTILES):
                    tensor.wait_ge(dve, k + 2)      # weights + round_k done
                    if k >= 2:
                        # psum buffer free once silu of tile k-2 done
                        tensor.wait_ge(act, k - 1)
                    ps = pss[k % 2]
                    xr = xrs[k % NB]
                    for c in range(NCHUNKS):
                        c0 = c * NC_CHUNK
                        for i in range(K):
                            mm = tensor.matmul(
                                ps[:, c0 : c0 + NC_CHUNK],
                                wdiag(k, i),
                                xr[:, c0 + i : c0 + i + NC_CHUNK],
                                start=(i == 0),
                                stop=(i == K - 1),
                                skip_group_check=True,
                            )
                    mm.then_inc(pe)

            @block.scalar
            def _(scalar):
                func = getattr(AF, _ACT_FUNC)
                for k in range(NTILES):
                    scalar.wait_ge(pe, k + 1)
                    if k >= NB:
                        # yt slot's previous store (tile k-NB) must be done
                        scalar.wait_ge(dout[k % NB], 16 * (k // NB))
                    scalar.activation(
                        out=yts[k % NB][:, :], in_=pss[k % 2][:, :],
                        func=func,
                        bias=0.0 if func == AF.Copy else zb,
                        scale=1.0,
                    ).then_inc(act)
                    # the DMA trigger races ahead of the still-streaming
                    # activation write; self-wait on its completion inc
                    scalar.wait_ge(act, k + 1)
                    r0, r1 = x_rows(k)
                    t0 = (k % NT) * T
                    scalar.dma_start(
                        out=y_d[r0:r1, t0 : t0 + T], in_=yts[k % NB][:, :]
                    ).then_inc(dout[k % NB], 16)
                for i in range(NB):
                    n_stores = len([k for k in range(NTILES) if k % NB == i])
                    scalar.wait_ge(dout[i], 16 * n_stores)

    return nc


def kernel(x, weight):
    global _last_results
    from concourse.bass_utils import run_bass_kernel_spmd

    x = np.asarray(x, dtype=np.float32)
    weight = np.asarray(weight, dtype=np.float32)

    nc = _build_program()

    in_maps = []
    for core in range(N_CORES):
        sl = slice(core * HC, (core + 1) * HC)
        # [B, S, HC] -> [B, HC, S] -> [ROWS, S] with 3 leading zero columns
        # (the causal padding), row r = b*HC + c
        xs = np.zeros((ROWS, S + 3), np.float32)
        xs[:, 3:] = x[:, :, sl].transpose(0, 2, 1).reshape(ROWS, S)
        ws = weight[sl, :]  # (HC, K)
        w_host = np.zeros((128, NU * K + 1), np.float32)
        for u in range(NU):
            blk = u % (HC // 128)
            w_host[:, u * K : (u + 1) * K] = ws[blk * 128 : (blk + 1) * 128, :]
        in_maps.append({"x": xs, "w": w_host})

    res = run_bass_kernel_spmd(nc, in_maps, list(range(N_CORES)))
    _last_results = res

    out = np.empty((B, S, H), np.float32)
    for core in range(N_CORES):
        sl = slice(core * HC, (core + 1) * HC)
        yc = res.results[core]["y"].reshape(B, HC, S)
        out[:, :, sl] = yc.transpose(0, 2, 1)
    return out
